# revision 7
# baseline (speedup 1.0000x reference)
"""Trainium2 Bass kernel for the batched constant-velocity Kalman filter.

Structure exploited:
  * The Kalman covariance recursion is data-independent: per-step gains and
    output stats (sx, sy, rho) are batch-wide scalars computed on host (the
    same scalars the estimation steps embed as instruction immediates).
  * Step 0 of the mean recursion is an exact identity (innovation == 0,
    pos_0 == z_1), so only 8 real estimation steps run on-device.
  * Outputs are three per-core DRAM tensors in partition-major planar
    layout (host gather is a pure byte permute + dtype cast):
      y_pos [p][t][2][j] fp16 -- per-trajectory positions
      y_sx  [p][t][2][j] fp8  -- (sx, sy) planes, batch-independent
      y_rho [p][t][j]    fp8  -- rho plane (exactly zero)
    Long contiguous runs per partition keep HWDGE descriptor generation
    (~15 ns/desc) far below the data time, unlike a [t,b,5] store.
  * The sx planes never touch a compute engine: they stream DRAM->DRAM
    from a host-staged constant; rho streams from a memset-once SBUF tile.
  * Estimation runs on DVE in contiguous-f16 ops (tensor_tensor 2x mode);
    prediction positions are an add-chain on DVE (one 2x TT per step),
    re-anchored every 6 steps (first anchor on DVE, rest on GpSimd) to
    bound f16 accumulation drift.
  * fp16/fp8 output precision fits the 2e-2 rel-err budget with >10x
    margin.

Sharding: pure data parallel over batch, B=131072 -> 16384 per core x 8.
Per-core batch shard maps to [128 partitions x 128 lanes], b = p*128 + j.
"""

import numpy as np

DT = 0.1
EPS = 0.01
N_CORES = 8
B_FULL = 131072
B_SHARD = B_FULL // N_CORES  # 16384
T_OBS = 10
P = 128                       # SBUF partitions
J = B_SHARD // P              # 128 lanes per partition
W = 2 * J                     # elems per obs step per partition ([zx|zy])
PBLK = 2 * J                  # pos/sx elems per step per partition
BLOCK = 6                     # pred steps per anchor block / DMA group


def _scalar_kalman(sigma_a, sigma_obs, sigma_init, n_est, len_pred):
    """Host-side data-independent 2x2 covariance recursion (float64)."""
    sa2 = float(sigma_a) ** 2
    r = float(sigma_obs) ** 2
    F = np.array([[1.0, DT], [0.0, 1.0]])
    Gm = np.array([DT * DT / 2.0, DT])
    Q = sa2 * np.outer(Gm, Gm)
    Pc = (float(sigma_init) ** 2) * np.eye(2)
    a_l, b_l, sx_l = [], [], []
    for _ in range(n_est):
        Pc = F @ Pc @ F.T + Q
        S = Pc[0, 0] + r
        a = Pc[0, 0] / S
        b = Pc[1, 0] / S
        IKH = np.array([[1.0 - a, 0.0], [-b, 1.0]])
        Pc = IKH @ Pc @ IKH.T + r * np.outer([a, b], [a, b])
        a_l.append(a)
        b_l.append(b)
        sx_l.append(np.sqrt(max(Pc[0, 0], EPS * EPS)))
    for _ in range(len_pred):
        Pc = F @ Pc @ F.T + Q
        sx_l.append(np.sqrt(max(Pc[0, 0], EPS * EPS)))
    return np.array(a_l), np.array(b_l), np.array(sx_l)


_CACHE = {}


def _build(sigma_a, sigma_obs, sigma_init, len_pred):
    import concourse.bacc as bacc
    import concourse.mybir as mybir
    import concourse.tile as tile

    OP = mybir.AluOpType
    F16 = mybir.dt.float16
    F8 = mybir.dt.float8e4
    U32 = mybir.dt.uint32

    n_est = T_OBS - 1
    n_out = n_est + len_pred
    a_g, b_g, _sx = _scalar_kalman(sigma_a, sigma_obs, sigma_init, n_est, len_pred)
    f32 = lambda z: float(np.float32(z))

    nc = bacc.Bacc(
        "TRN2",
        target_bir_lowering=False,
        debug=False,
        enable_asserts=False,
        num_devices=N_CORES,
    )
    x = nc.dram_tensor("x", [P, T_OBS * W], F16, kind="ExternalInput")
    fm = nc.dram_tensor("fm", [P, n_out * PBLK], F8, kind="ExternalInput")
    y_pos = nc.dram_tensor("y_pos", [P, n_out * PBLK], F16, kind="ExternalOutput")
    y_sx = nc.dram_tensor("y_sx", [P, n_out * PBLK], F8, kind="ExternalOutput")
    y_rho = nc.dram_tensor("y_rho", [P, n_out * J], F8, kind="ExternalOutput")
    x_ap = x.ap()
    fm_ap = fm.ap()
    ypos_ap = y_pos.ap()
    ysx_ap = y_sx.ap()
    yrho_ap = y_rho.ap()

    with tile.TileContext(nc) as tc:
        with (
            tc.tile_pool(name="zp", bufs=1) as zp,
            tc.tile_pool(name="sp", bufs=1) as sp,
        ):
            zt = zp.tile([P, T_OBS * W], F16, name="zt")
            zr = zp.tile([P, n_out * J], F8, name="zr")      # rho zeros
            ep = sp.tile([P, n_est * PBLK], F16, name="ep")   # est positions
            qp = sp.tile([P, len_pred * PBLK], F16, name="qp")  # pred positions
            vdt = sp.tile([P, W], F16, name="vdt")
            pp = sp.tile([P, W], F16, name="pp")
            ixy = sp.tile([P, W], F16, name="ixy")

            # --- input DMAs on SWDGE (Pool dispatch beats HWDGE to the
            # engines, ahead of the background d2d stream) ---
            nc.gpsimd.dma_start(zt[:, : 3 * W], x_ap[:, : 3 * W])
            nc.gpsimd.dma_start(zt[:, 3 * W :], x_ap[:, 3 * W :])

            # rho zeros (one u32 memset while DVE waits for input)
            nc.vector.memset(zr.bitcast(U32), 0)
            nc.gpsimd.dma_start(yrho_ap, zr)

            # sx planes: background DRAM->DRAM stream on the HWDGE rings
            half = (n_out // 2) * PBLK
            nc.sync.dma_start(ysx_ap[:, :half], fm_ap[:, :half])
            nc.scalar.dma_start(ysx_ap[:, half:], fm_ap[:, half:])

            def zv(s):
                return zt[:, s * W : (s + 1) * W]

            def epos(t):
                return ep[:, t * PBLK : (t + 1) * PBLK]

            def qpos(i):
                return qp[:, i * PBLK : (i + 1) * PBLK]

            stt = nc.vector.scalar_tensor_tensor
            rings = [nc.sync, nc.scalar]
            ring_i = 1

            # --- estimation: pos_0 = z1 exactly; 8 real steps ---
            nc.vector.tensor_sub(vdt, zv(1), zv(0))     # v*dt (exact)
            nc.vector.tensor_copy(epos(0), zv(1))
            for t in range(1, n_est):
                prev = zv(1) if t == 1 else epos(t - 1)
                nc.vector.tensor_add(pp, vdt, prev)
                nc.vector.tensor_sub(ixy, zv(t + 1), pp)
                stt(epos(t), ixy, f32(a_g[t]), pp, OP.mult, OP.add)
                stt(vdt, ixy, f32(b_g[t] * DT), vdt, OP.mult, OP.add)
                if t == 4:
                    rings[0].dma_start(ypos_ap[:, : 5 * PBLK], ep[:, : 5 * PBLK])
            rings[1].dma_start(
                ypos_ap[:, 5 * PBLK : n_est * PBLK], ep[:, 5 * PBLK :]
            )

            # --- prediction: first block per-step, then 6-wide chained
            # block adds of a tiled 6*vdt increment ---
            pos9 = epos(n_est - 1)
            assert len_pred % BLOCK == 0
            n_blk = len_pred // BLOCK
            # vdt6 tiled BLOCK times (built during est, cheap 4x copies)
            vdt6 = sp.tile([P, BLOCK * W], F16, name="vdt6")
            nc.vector.tensor_scalar_mul(vdt6[:, :W], vdt, f32(BLOCK))
            for r in range(1, BLOCK):
                nc.vector.tensor_copy(vdt6[:, r * W : (r + 1) * W], vdt6[:, :W])

            stt(qpos(0), vdt, 1.0, pos9, OP.mult, OP.add)
            for i in range(1, BLOCK):
                nc.vector.tensor_add(qpos(i), qpos(i - 1), vdt)
            rings[0].dma_start(
                ypos_ap[:, n_est * PBLK : (n_est + BLOCK) * PBLK],
                qp[:, : BLOCK * PBLK],
            )
            for b in range(1, n_blk):
                lo, hi = b * BLOCK * PBLK, (b + 1) * BLOCK * PBLK
                nc.vector.tensor_add(
                    qp[:, lo:hi], qp[:, lo - BLOCK * PBLK : lo], vdt6
                )
                # split the last block's store so the final drain is short
                if b == n_blk - 1:
                    mid = lo + (BLOCK // 2) * PBLK
                    rings[1].dma_start(
                        ypos_ap[:, (n_est * PBLK) + lo : (n_est * PBLK) + mid],
                        qp[:, lo:mid],
                    )
                    rings[0].dma_start(
                        ypos_ap[:, (n_est * PBLK) + mid : (n_est * PBLK) + hi],
                        qp[:, mid:hi],
                    )
                else:
                    rings[(b + 1) % 2].dma_start(
                        ypos_ap[:, (n_est * PBLK) + lo : (n_est * PBLK) + hi],
                        qp[:, lo:hi],
                    )

    nc.compile()
    return nc


def _make_in_maps(x_full, sigma_a=0.5, sigma_obs=0.3, sigma_init=1.0,
                  len_pred=30):
    """Full [10, B, 2] f32 -> per-core input dict."""
    import ml_dtypes

    n_est = T_OBS - 1
    n_out = n_est + len_pred
    x6 = np.asarray(x_full, dtype=np.float32).reshape(T_OBS, N_CORES, P, J, 2)
    # (s, core, p, j, c) -> (core, p, s, c, j)
    xt = np.ascontiguousarray(x6.transpose(1, 2, 0, 4, 3)).astype(np.float16)
    _, _, sx_g = _scalar_kalman(sigma_a, sigma_obs, sigma_init, n_est, len_pred)
    row = np.repeat(sx_g.astype(np.float32), PBLK).astype(ml_dtypes.float8_e4m3)
    fm = np.ascontiguousarray(np.broadcast_to(row, (P, n_out * PBLK)))
    return [
        {"x": xt[c].reshape(P, T_OBS * W), "fm": fm}
        for c in range(N_CORES)
    ]


def _gather_out(results, len_pred):
    """Per-core y_pos/y_sx/y_rho -> full [n_out, B, 5] f32."""
    n_out = T_OBS - 1 + len_pred
    pos = np.stack([r["y_pos"] for r in results]).reshape(
        N_CORES, P, n_out, 2, J)
    sx = np.stack([r["y_sx"] for r in results]).reshape(
        N_CORES, P, n_out, 2, J).astype(np.float16)
    rho = np.stack([r["y_rho"] for r in results]).reshape(
        N_CORES, P, n_out, J).astype(np.float16)
    full = np.empty((n_out, B_FULL, 5), np.float32)
    fullv = full.reshape(n_out, N_CORES, P, J, 5)
    for c in range(2):
        fullv[:, :, :, :, c] = pos[:, :, :, c, :].transpose(2, 0, 1, 3)
        fullv[:, :, :, :, 2 + c] = sx[:, :, :, c, :].transpose(2, 0, 1, 3)
    fullv[:, :, :, :, 4] = rho.transpose(2, 0, 1, 3)
    return full


def kernel(**inputs):
    from concourse import bass_utils

    x_full = np.asarray(inputs["inputs"], dtype=np.float32)
    sigma_a = float(np.asarray(inputs["sigma_a"]))
    sigma_obs = float(np.asarray(inputs["sigma_obs"]))
    sigma_init = float(np.asarray(inputs["sigma_init"]))
    len_pred = int(np.asarray(inputs["len_pred"]))
    assert x_full.shape == (T_OBS, B_FULL, 2), x_full.shape

    key = (sigma_a, sigma_obs, sigma_init, len_pred)
    if key not in _CACHE:
        _CACHE[key] = _build(sigma_a, sigma_obs, sigma_init, len_pred)
    nc = _CACHE[key]

    in_maps = _make_in_maps(x_full, sigma_a, sigma_obs, sigma_init, len_pred)
    res = bass_utils.run_bass_kernel_spmd(nc, in_maps, core_ids=list(range(N_CORES)))
    return _gather_out(res.results, len_pred)


if __name__ == "__main__":
    import ref_np

    inp = ref_np.setup_inputs_np()
    out = kernel(**inp)
    exp = ref_np.reference_np(**inp)
    err = np.abs(out - exp)
    print("max abs err:", err.max(), " rel:", err.max() / np.abs(exp).max())
